# revision 1
# baseline (speedup 1.0000x reference)
"""DeepFM forward on 8 Trainium2 NeuronCores (Bass/Tile, SPMD).

Strategy: data-parallel over the batch (2048 rows/core), embedding tables
replicated. The first-order and second-order cat tables are fused host-side
into one [F_CAT*V, 65] table so a single indirect-DMA gather per batch tile
fetches both. The MLP runs in bf16 (fp32 accumulation in PSUM); batchnorm
statistics are exchanged with two tiny AllReduces. FM arithmetic stays fp32.

Layout: MLP operands are kept feature-major ("X.T": [feat, batch]) so the
contraction dim sits on SBUF partitions; gathered rows are transposed with
the DMA xbar (bf16). FM terms are computed in row layout during the gather
phase; the final logit is assembled in row layout via matmuls with the
hidden activations as the stationary operand.
"""

import numpy as np

# ---- problem constants (hardcoded per harness contract) ----
B, F_CAT, F_CONT, V, D = 16384, 26, 13, 100000, 64
H1, H2 = 1024, 512
N_CORES = 8
BN_EPS = 1e-5

CFG_FULL = dict(B=B, V=V, n_cores=N_CORES)

_P = 128


def _build_program(cfg):
    """Build the per-core SPMD Bass program. Returns (nc, names)."""
    import concourse.bacc as bacc
    import concourse.bass as bass
    import concourse.mybir as mybir
    import concourse.tile as tile
    from concourse.masks import make_identity

    F32, BF16, I32 = mybir.dt.float32, mybir.dt.float16, mybir.dt.int32
    AF = mybir.ActivationFunctionType
    OP = mybir.AluOpType
    AX = mybir.AxisListType
    P = _P

    ncore = cfg["n_cores"]
    Bfull = cfg["B"]
    Vv = cfg["V"]
    Bc = Bfull // ncore          # batch rows per core
    TB = Bc // P                 # batch tiles per core
    NB = min(512, Bc)            # matmul moving free dim
    NN = Bc // NB                # batch n-tiles
    TPN = NB // P                # 128-tiles per n-tile
    KC = F_CAT * D // P          # cat K-chunks (13)
    NKC = KC + 1                 # + cont chunk
    NM1 = H1 // P                # 8
    NM2 = H2 // P                # 4
    EW = D + 1                   # gathered row width (64 emb + 1 first-order)
    RW = F_CAT * EW              # gathered row bytes/4 per batch row (1690)
    rg = [list(range(ncore))]

    NQ = cfg.get("swdge_queues", 4)
    gqn = [0]
    nc = bacc.Bacc(num_devices=ncore, num_swdge_queues=NQ)

    idxg = nc.dram_tensor("idxg", [Bc, F_CAT], I32, kind="ExternalInput")
    cfin = nc.dram_tensor("cfin", [Bc, F_CONT], F32, kind="ExternalInput")
    bigt = nc.dram_tensor("bigt", [F_CAT * Vv, EW], F32, kind="ExternalInput")
    w1 = nc.dram_tensor("w1", [NKC * P, H1], BF16, kind="ExternalInput")
    w2 = nc.dram_tensor("w2", [H1, H2], BF16, kind="ExternalInput")
    w3 = nc.dram_tensor("w3", [P, NM2], BF16, kind="ExternalInput")
    ct2 = nc.dram_tensor("ct2", [F_CONT, D], F32, kind="ExternalInput")
    cmisc = nc.dram_tensor("cmisc", [P, 2 * F_CONT], F32, kind="ExternalInput")
    bnp = nc.dram_tensor("bnp", [P, 3 * NM1 + 3 * NM2 + 1], F32, kind="ExternalInput")
    out = nc.dram_tensor("out", [P, 2 * TB], F32, kind="ExternalOutput")

    with tile.TileContext(nc) as tc:
        with (
            tc.tile_pool(name="const", bufs=1) as cpool,
            tc.tile_pool(name="big", bufs=1) as bpool,
            tc.tile_pool(name="work", bufs=2) as wpool,
            tc.tile_pool(name="psmm", bufs=4, space="PSUM") as psmm,
            tc.tile_pool(name="pssm", bufs=4, space="PSUM") as pssm,
            tc.tile_pool(name="dram", bufs=1, space="DRAM") as dpool,
        ):
            # ---- constants ----
            w1sb = []
            for k in range(NKC):
                t = cpool.tile([P, H1], BF16, tag=f"w1_{k}")
                nc.sync.dma_start(out=t[:], in_=w1[k * P : (k + 1) * P, :])
                w1sb.append(t)
            w2sb = []
            for k in range(NM1):
                t = cpool.tile([P, H2], BF16, tag=f"w2_{k}")
                nc.sync.dma_start(out=t[:], in_=w2[k * P : (k + 1) * P, :])
                w2sb.append(t)
            w3sb = cpool.tile([P, NM2], BF16, tag="w3")
            nc.sync.dma_start(out=w3sb[:], in_=w3[:])
            ct2sb = cpool.tile([F_CONT, D], F32, tag="ct2")
            nc.sync.dma_start(out=ct2sb[:], in_=ct2[:])
            cmsb = cpool.tile([P, 2 * F_CONT], F32, tag="cmisc")
            nc.sync.dma_start(out=cmsb[:], in_=cmisc[:])
            bnsb = cpool.tile([P, 3 * NM1 + 3 * NM2 + 1], F32, tag="bnp")
            nc.sync.dma_start(out=bnsb[:], in_=bnp[:])
            ident = cpool.tile([P, P], F32, tag="ident")
            make_identity(nc, ident[:])
            eps_t = cpool.tile([P, 1], F32, tag="eps")
            nc.vector.memset(eps_t[:], BN_EPS)

            b1c = bnsb[:, 0:NM1]
            g1c = bnsb[:, NM1 : 2 * NM1]
            be1c = bnsb[:, 2 * NM1 : 3 * NM1]
            o2 = 3 * NM1
            b2c = bnsb[:, o2 : o2 + NM2]
            g2c = bnsb[:, o2 + NM2 : o2 + 2 * NM2]
            be2c = bnsb[:, o2 + 2 * NM2 : o2 + 3 * NM2]
            bias_col = bnsb[:, o2 + 3 * NM2 : o2 + 3 * NM2 + 1]
            t1b = cmsb[:, 0:F_CONT]
            rb = cmsb[:, F_CONT : 2 * F_CONT]

            # ---- persistent activations ----
            xtn = [
                bpool.tile([P, NKC, NB], BF16, tag=f"xtn_{n}", name=f"xtn_{n}")
                for n in range(NN)
            ]
            for n in range(NN):
                nc.vector.memset(xtn[n][:, KC, :], 0.0)
            h1t = [bpool.tile([P, Bc], BF16, tag=f"h1_{m}", name=f"h1_{m}") for m in range(NM1)]
            h2t = [bpool.tile([P, Bc], BF16, tag=f"h2_{m}", name=f"h2_{m}") for m in range(NM2)]

            # FM accumulators (col per batch tile)
            qcat = bpool.tile([P, TB], F32, tag="qcat")
            q2t = bpool.tile([P, TB], F32, tag="q2t")
            qct = bpool.tile([P, TB], F32, tag="qct")
            f1t = bpool.tile([P, TB], F32, tag="f1t")
            fct = bpool.tile([P, TB], F32, tag="fct")
            fm_all = bpool.tile([P, TB], F32, tag="fm")
            acc1 = bpool.tile([P, NM1 * NN], F32, tag="acc1")
            acc1s = bpool.tile([P, NM1 * NN], F32, tag="acc1s")
            acc2 = bpool.tile([P, NM2 * NN], F32, tag="acc2")
            acc2s = bpool.tile([P, NM2 * NN], F32, tag="acc2s")
            scr = bpool.tile([P, 64], F32, tag="scr")
            scrh = bpool.tile([P, 2048], BF16, tag="scrh")
            out_sb = bpool.tile([P, 2 * TB], F32, tag="outsb")

            # ---- phase A: gather + FM + transpose ----
            for t in range(TB):
                n, tp = t // TPN, t % TPN
                idx_t = wpool.tile([P, F_CAT], I32, tag="idx")
                nc.sync.dma_start(out=idx_t[:], in_=idxg[t * P : (t + 1) * P, :])
                rows = wpool.tile([P, RW], F32, tag="rows")
                for f in range(F_CAT):
                    inst = nc.gpsimd.indirect_dma_start(
                        out=rows[:, f * EW : (f + 1) * EW],
                        out_offset=None,
                        in_=bigt[:],
                        in_offset=bass.IndirectOffsetOnAxis(
                            ap=idx_t[:, f : f + 1], axis=0
                        ),
                    )
                    if NQ > 1:
                        inst.ins.queue = f"qPoolDynamic{(gqn[0] % NQ) or ''}"
                        gqn[0] += 1
                cf_t = wpool.tile([P, F_CONT], F32, tag="cf")
                nc.sync.dma_start(out=cf_t[:], in_=cfin[t * P : (t + 1) * P, :])

                rows_fe = rows[:].rearrange("p (f e) -> p f e", e=EW)
                cat3 = rows_fe[:, :, :D]            # [P, 26, 64]
                # cast cat cols to fp16 row-major (feeds transpose and q)
                xrow = wpool.tile([P, F_CAT * D], BF16, tag="xrow")
                nc.scalar.activation(
                    out=xrow[:].rearrange("p (f e) -> p f e", e=D),
                    in_=cat3, func=AF.Copy,
                )
                # q_cat = sum E^2 from the fp16 copy; scrh is a dummy output
                nc.scalar.activation(
                    out=scrh[:, : F_CAT * D], in_=xrow[:], func=AF.Square,
                    accum_out=qcat[:, t : t + 1],
                )
                # s = sum_f E  (keep d): [P, 64]
                s_t = wpool.tile([P, D], F32, tag="s")
                cat_df = rows[:].rearrange("p (f e) -> p e f", e=EW)[:, :D, :]
                nc.vector.tensor_reduce(
                    out=s_t[:], in_=cat_df, axis=AX.X, op=OP.add
                )
                # first-order cat: sum of col 64 of each block
                nc.vector.tensor_reduce(
                    out=f1t[:, t : t + 1],
                    in_=rows_fe[:, :, D : D + 1].rearrange("p f e -> p e f"),
                    axis=AX.X, op=OP.add,
                )
                # cont: transpose cf tile -> [13, P]
                tr_ps = pssm.tile([F_CONT, P], F32, tag="sm")
                nc.tensor.transpose(out=tr_ps[:], in_=cf_t[:], identity=ident[:])
                cfT = wpool.tile([F_CONT, P], F32, tag="cfT")
                nc.vector.tensor_copy(out=cfT[:], in_=tr_ps[:])
                # cont block of X.T (bf16) goes into the last K chunk
                nc.vector.tensor_copy(
                    out=xtn[n][0:F_CONT, KC, tp * P : (tp + 1) * P], in_=tr_ps[:]
                )
                # s_cont = cfT.T @ ct2 : [P, 64]
                ss_ps = pssm.tile([P, D], F32, tag="sm")
                nc.tensor.matmul(
                    out=ss_ps[:], lhsT=cfT[:], rhs=ct2sb[:], start=True, stop=True
                )
                nc.vector.tensor_tensor(
                    out=s_t[:], in0=s_t[:], in1=ss_ps[:], op=OP.add
                )
                # q2 = sum_d s^2
                nc.scalar.activation(
                    out=scr[:, :D], in_=s_t[:], func=AF.Square,
                    accum_out=q2t[:, t : t + 1],
                )
                # cont second-order: qc = sum_f cf^2 * r ; first-order fc
                c13a = wpool.tile([P, F_CONT], F32, tag="c13a")
                c13b = wpool.tile([P, F_CONT], F32, tag="c13b")
                nc.vector.tensor_tensor(out=c13a[:], in0=cf_t[:], in1=rb, op=OP.mult)
                nc.vector.tensor_tensor(out=c13b[:], in0=c13a[:], in1=cf_t[:], op=OP.mult)
                nc.vector.tensor_reduce(
                    out=qct[:, t : t + 1], in_=c13b[:], axis=AX.X, op=OP.add
                )
                nc.vector.tensor_tensor(out=c13a[:], in0=cf_t[:], in1=t1b, op=OP.mult)
                nc.vector.tensor_reduce(
                    out=fct[:, t : t + 1], in_=c13a[:], axis=AX.X, op=OP.add
                )
                # DMA-transpose (blocked 3D dest) into X.T chunks
                nc.sync.dma_start_transpose(
                    out=xtn[n][:, 0:KC, tp * P : (tp + 1) * P],
                    in_=xrow[:],
                )

            # fm = 0.5*(q2 - qcat - qc) + f1 + fc
            nc.vector.tensor_tensor(out=fm_all[:], in0=qcat[:], in1=qct[:], op=OP.add)
            nc.vector.tensor_tensor(out=fm_all[:], in0=q2t[:], in1=fm_all[:], op=OP.subtract)
            nc.vector.tensor_scalar(
                out=fm_all[:], in0=fm_all[:], scalar1=0.5, scalar2=None, op0=OP.mult
            )
            nc.vector.tensor_tensor(out=fm_all[:], in0=fm_all[:], in1=f1t[:], op=OP.add)
            nc.vector.tensor_tensor(out=fm_all[:], in0=fm_all[:], in1=fct[:], op=OP.add)

            # ---- phase B: layer 1 matmul ----
            for n in range(NN):
                for m in range(NM1):
                    ps = psmm.tile([P, NB], F32, tag="mm")
                    for k in range(NKC):
                        nc.tensor.matmul(
                            out=ps[:],
                            lhsT=w1sb[k][:, m * P : (m + 1) * P],
                            rhs=xtn[n][:, k, :],
                            start=(k == 0),
                            stop=(k == NKC - 1),
                        )
                    j = m * NN + n
                    nc.scalar.activation(
                        out=h1t[m][:, n * NB : (n + 1) * NB], in_=ps[:],
                        func=AF.Identity, bias=b1c[:, m : m + 1],
                        accum_out=acc1[:, j : j + 1],
                    )
                    nc.scalar.activation(
                        out=scrh[:, :NB], in_=h1t[m][:, n * NB : (n + 1) * NB],
                        func=AF.Square,
                        accum_out=acc1s[:, j : j + 1],
                    )

            # ---- phase C: BN1 (AllReduce stats) ----
            st1 = bpool.tile([P, 2 * NM1], F32, tag="st1")
            nc.vector.tensor_reduce(
                out=st1[:, :NM1],
                in_=acc1[:].rearrange("p (m n) -> p m n", n=NN),
                axis=AX.X, op=OP.add,
            )
            nc.vector.tensor_reduce(
                out=st1[:, NM1:],
                in_=acc1s[:].rearrange("p (m n) -> p m n", n=NN),
                axis=AX.X, op=OP.add,
            )
            st1i = dpool.tile([P, 2 * NM1], F32, tag="st1i")
            st1o = dpool.tile([P, 2 * NM1], F32, tag="st1o")
            nc.gpsimd.dma_start(out=st1i[:], in_=st1[:])
            nc.gpsimd.collective_compute(
                "AllReduce", OP.add, replica_groups=rg,
                ins=[st1i[:].opt()], outs=[st1o[:].opt()],
            )
            gst1 = bpool.tile([P, 2 * NM1], F32, tag="gst1")
            nc.gpsimd.dma_start(out=gst1[:], in_=st1o[:])

            mu1 = bpool.tile([P, NM1], F32, tag="mu1")
            var1 = bpool.tile([P, NM1], F32, tag="var1")
            a1 = bpool.tile([P, NM1], F32, tag="a1")
            bp1 = bpool.tile([P, NM1], F32, tag="bp1")
            inv_b = 1.0 / Bfull
            nc.vector.tensor_scalar(
                out=mu1[:], in0=gst1[:, :NM1], scalar1=inv_b, scalar2=None, op0=OP.mult
            )
            nc.vector.tensor_tensor(out=var1[:], in0=mu1[:], in1=mu1[:], op=OP.mult)
            nc.vector.tensor_scalar(
                out=a1[:], in0=gst1[:, NM1:], scalar1=inv_b, scalar2=None, op0=OP.mult
            )
            nc.vector.tensor_tensor(out=var1[:], in0=a1[:], in1=var1[:], op=OP.subtract)
            nc.scalar.activation(
                out=var1[:], in_=var1[:], func=AF.Sqrt, bias=eps_t[:, 0:1]
            )
            nc.vector.reciprocal(out=var1[:], in_=var1[:])
            nc.vector.tensor_tensor(out=a1[:], in0=g1c, in1=var1[:], op=OP.mult)
            nc.vector.tensor_tensor(out=bp1[:], in0=mu1[:], in1=a1[:], op=OP.mult)
            nc.vector.tensor_tensor(out=bp1[:], in0=be1c, in1=bp1[:], op=OP.subtract)
            for m in range(NM1):
                nc.scalar.activation(
                    out=h1t[m][:], in_=h1t[m][:], func=AF.Relu,
                    scale=a1[:, m : m + 1], bias=bp1[:, m : m + 1],
                )

            # ---- phase D: layer 2 ----
            for n in range(NN):
                for m in range(NM2):
                    ps = psmm.tile([P, NB], F32, tag="mm")
                    for k in range(NM1):
                        nc.tensor.matmul(
                            out=ps[:],
                            lhsT=w2sb[k][:, m * P : (m + 1) * P],
                            rhs=h1t[k][:, n * NB : (n + 1) * NB],
                            start=(k == 0),
                            stop=(k == NM1 - 1),
                        )
                    j = m * NN + n
                    nc.scalar.activation(
                        out=h2t[m][:, n * NB : (n + 1) * NB], in_=ps[:],
                        func=AF.Identity, bias=b2c[:, m : m + 1],
                        accum_out=acc2[:, j : j + 1],
                    )
                    nc.scalar.activation(
                        out=scrh[:, :NB], in_=h2t[m][:, n * NB : (n + 1) * NB],
                        func=AF.Square,
                        accum_out=acc2s[:, j : j + 1],
                    )

            # ---- phase E: BN2 ----
            st2 = bpool.tile([P, 2 * NM2], F32, tag="st2")
            nc.vector.tensor_reduce(
                out=st2[:, :NM2],
                in_=acc2[:].rearrange("p (m n) -> p m n", n=NN),
                axis=AX.X, op=OP.add,
            )
            nc.vector.tensor_reduce(
                out=st2[:, NM2:],
                in_=acc2s[:].rearrange("p (m n) -> p m n", n=NN),
                axis=AX.X, op=OP.add,
            )
            st2i = dpool.tile([P, 2 * NM2], F32, tag="st2i")
            st2o = dpool.tile([P, 2 * NM2], F32, tag="st2o")
            nc.gpsimd.dma_start(out=st2i[:], in_=st2[:])
            nc.gpsimd.collective_compute(
                "AllReduce", OP.add, replica_groups=rg,
                ins=[st2i[:].opt()], outs=[st2o[:].opt()],
            )
            gst2 = bpool.tile([P, 2 * NM2], F32, tag="gst2")
            nc.gpsimd.dma_start(out=gst2[:], in_=st2o[:])

            mu2 = bpool.tile([P, NM2], F32, tag="mu2")
            var2 = bpool.tile([P, NM2], F32, tag="var2")
            a2 = bpool.tile([P, NM2], F32, tag="a2")
            bp2 = bpool.tile([P, NM2], F32, tag="bp2")
            nc.vector.tensor_scalar(
                out=mu2[:], in0=gst2[:, :NM2], scalar1=inv_b, scalar2=None, op0=OP.mult
            )
            nc.vector.tensor_tensor(out=var2[:], in0=mu2[:], in1=mu2[:], op=OP.mult)
            nc.vector.tensor_scalar(
                out=a2[:], in0=gst2[:, NM2:], scalar1=inv_b, scalar2=None, op0=OP.mult
            )
            nc.vector.tensor_tensor(out=var2[:], in0=a2[:], in1=var2[:], op=OP.subtract)
            nc.scalar.activation(
                out=var2[:], in_=var2[:], func=AF.Sqrt, bias=eps_t[:, 0:1]
            )
            nc.vector.reciprocal(out=var2[:], in_=var2[:])
            nc.vector.tensor_tensor(out=a2[:], in0=g2c, in1=var2[:], op=OP.mult)
            nc.vector.tensor_tensor(out=bp2[:], in0=mu2[:], in1=a2[:], op=OP.mult)
            nc.vector.tensor_tensor(out=bp2[:], in0=be2c, in1=bp2[:], op=OP.subtract)
            for m in range(NM2):
                nc.scalar.activation(
                    out=h2t[m][:], in_=h2t[m][:], func=AF.Relu,
                    scale=a2[:, m : m + 1], bias=bp2[:, m : m + 1],
                )

            # ---- phase F: layer 3 + sigmoid + output ----
            for t in range(TB):
                psd = pssm.tile([P, 1], F32, tag="sm")
                for c in range(NM2):
                    nc.tensor.matmul(
                        out=psd[:],
                        lhsT=h2t[c][:, t * P : (t + 1) * P],
                        rhs=w3sb[:, c : c + 1],
                        start=(c == 0),
                        stop=(c == NM2 - 1),
                    )
                zt = wpool.tile([P, 1], F32, tag="zt")
                nc.vector.tensor_tensor(
                    out=zt[:], in0=fm_all[:, t : t + 1], in1=psd[:], op=OP.add
                )
                nc.scalar.activation(
                    out=out_sb[:, 2 * t + 1 : 2 * t + 2], in_=zt[:],
                    func=AF.Sigmoid, bias=bias_col,
                )
                nc.scalar.activation(
                    out=out_sb[:, 2 * t : 2 * t + 1],
                    in_=out_sb[:, 2 * t + 1 : 2 * t + 2],
                    func=AF.Copy, bias=1.0, scale=-1.0,
                )
            nc.sync.dma_start(out=out[:], in_=out_sb[:])

    return nc


def _prep_shared(inputs, cfg):
    """Host-side parameter prep (batch-independent). Returns dict of arrays
    shared by all cores."""
    import ml_dtypes

    Vv = cfg["V"]
    f32 = np.float32
    cat_t1 = np.asarray(inputs["cat_t1"], f32)          # [26, V]
    cat_t2 = np.asarray(inputs["cat_t2"], f32)          # [26, V, 64]
    cont_t1 = np.asarray(inputs["cont_t1"], f32)        # [13]
    cont_t2 = np.asarray(inputs["cont_t2"], f32)        # [13, 64]
    W1 = np.asarray(inputs["W1"], f32)                  # [2496, 1024]
    W2 = np.asarray(inputs["W2"], f32)
    W3 = np.asarray(inputs["W3"], f32)                  # [512, 1]
    b1 = np.asarray(inputs["b1"], f32)
    g1 = np.asarray(inputs["g1"], f32)
    be1 = np.asarray(inputs["be1"], f32)
    b2 = np.asarray(inputs["b2"], f32)
    g2 = np.asarray(inputs["g2"], f32)
    be2 = np.asarray(inputs["be2"], f32)
    b3 = np.asarray(inputs["b3"], f32)
    bias = np.asarray(inputs["bias"], f32)

    EW = D + 1
    bigt = np.empty((F_CAT * Vv, EW), f32)
    bigt[:, :D] = cat_t2.reshape(F_CAT * Vv, D)
    bigt[:, D] = cat_t1.reshape(F_CAT * Vv)

    ncat = F_CAT * D  # 1664
    NKC = ncat // _P + 1
    W1eff = np.einsum("fd,fdh->fh", cont_t2, W1[ncat:].reshape(F_CONT, D, H1))
    # cont rows (folded through cont_t2) live at the start of the last K chunk
    w1p = np.zeros((NKC * _P, H1), f32)
    w1p[:ncat] = W1[:ncat]
    w1p[ncat : ncat + F_CONT] = W1eff
    bf16 = np.float16

    NM1, NM2 = H1 // _P, H2 // _P
    bnp = np.zeros((_P, 3 * NM1 + 3 * NM2 + 1), f32)
    bnp[:, 0:NM1] = b1.reshape(NM1, _P).T
    bnp[:, NM1 : 2 * NM1] = g1.reshape(NM1, _P).T
    bnp[:, 2 * NM1 : 3 * NM1] = be1.reshape(NM1, _P).T
    o2 = 3 * NM1
    bnp[:, o2 : o2 + NM2] = b2.reshape(NM2, _P).T
    bnp[:, o2 + NM2 : o2 + 2 * NM2] = g2.reshape(NM2, _P).T
    bnp[:, o2 + 2 * NM2 : o2 + 3 * NM2] = be2.reshape(NM2, _P).T
    bnp[:, o2 + 3 * NM2] = float(bias[0]) + float(b3[0])

    cmisc = np.zeros((_P, 2 * F_CONT), f32)
    cmisc[:, :F_CONT] = cont_t1[None, :]
    cmisc[:, F_CONT:] = (cont_t2**2).sum(axis=1)[None, :]

    return {
        "bigt": bigt,
        "w1": w1p.astype(bf16),
        "w2": W2.astype(bf16),
        "w3": W3[:, 0].reshape(NM2, _P).T.astype(bf16).copy(),
        "ct2": cont_t2,
        "cmisc": cmisc,
        "bnp": bnp,
    }


def _prep_in_maps(inputs, cfg):
    """Build the per-core input maps (shard batch, replicate params)."""
    ncore = cfg["n_cores"]
    Vv = cfg["V"]
    Bc = cfg["B"] // ncore
    shared = _prep_shared(inputs, cfg)
    cat = np.asarray(inputs["cat_feats"]).astype(np.int32)
    cont = np.asarray(inputs["cont_feats"], np.float32)
    idxg = cat + (np.arange(F_CAT, dtype=np.int32) * Vv)[None, :]
    in_maps = []
    for c in range(ncore):
        m = dict(shared)
        m["idxg"] = idxg[c * Bc : (c + 1) * Bc]
        m["cfin"] = cont[c * Bc : (c + 1) * Bc]
        in_maps.append(m)
    return in_maps


def _unshard(results, cfg):
    ncore = cfg["n_cores"]
    Bc = cfg["B"] // ncore
    TB = Bc // _P
    outs = []
    for c in range(ncore):
        a = results[c]["out"]  # [128, 2*TB]
        outs.append(a.reshape(_P, TB, 2).transpose(1, 0, 2).reshape(Bc, 2))
    return np.concatenate(outs, axis=0)


_CACHE = {}


def _get_program(cfg_key):
    if cfg_key not in _CACHE:
        cfg = dict(B=cfg_key[0], V=cfg_key[1], n_cores=cfg_key[2])
        nc = _build_program(cfg)
        nc.finalize()
        _CACHE[cfg_key] = nc
    return _CACHE[cfg_key]


def run(inputs, trace=False, cfg=None):
    from concourse import bass_utils

    cfg = cfg or CFG_FULL
    nc = _get_program((cfg["B"], cfg["V"], cfg["n_cores"]))
    in_maps = _prep_in_maps(inputs, cfg)
    res = bass_utils.run_bass_kernel_spmd(
        nc, in_maps, core_ids=list(range(cfg["n_cores"])), trace=trace
    )
    return _unshard(res.results, cfg), res


def kernel(**inputs) -> np.ndarray:
    out, _ = run(inputs, trace=False)
    return out



# revision 7
# speedup vs baseline: 1.0028x; 1.0028x over previous
"""DeepFM forward on 8 Trainium2 NeuronCores (Bass/Tile, SPMD).

v2: embedding lookups via the custom dma_gather instruction (vectorized
SWDGE descriptor generation, ~1us + 0.34ns/desc per instruction) instead
of per-feature INDIRECT1D (~1.1us per 128 rows).

Table layout: fp16, one 512B row per (feature, vocab-pair): two packed
entries [emb(64) | t1 | pad(63)] each. Index = (v >> 1) - 17232 fits a
single signed int16 window per feature; entry parity selected on-chip.
Per feature, three gathers (NI=1024,1024,384) with a one-chunk overlap
scheme: each instruction's final dst chunk is never consumed (trailing
negative-index trim can only damage that chunk), the following gather
re-covers it, and the last gather ends in positive pad indices.

Batch data-parallel over cores (2048 rows/core); MLP in fp16 with fp32
PSUM accumulation; BN stats exchanged with two small AllReduces; W1
streamed from HBM in [128,512] slices (SBUF pressure).
"""

import numpy as np

# ---- problem constants (hardcoded per harness contract) ----
B, F_CAT, F_CONT, V, D = 16384, 26, 13, 100000, 64
H1, H2 = 1024, 512
N_CORES = 8
BN_EPS = 1e-5

CFG_FULL = dict(B=B, V=V, n_cores=N_CORES)

_P = 128
VH = V // 2            # 50000 packed pair-rows per feature
WOFF = 17232           # idx' = (v>>1) - WOFF  in [-17232, 32767]
EP = 256               # fp16 elements per packed row (512B)
IDXF = 64 + 64 + 24    # idx free-dim per feature (A,B,C blocks)


def _build_program(cfg):
    import concourse.bacc as bacc
    import concourse.bass as bass
    import concourse.mybir as mybir
    import concourse.tile as tile
    from concourse.masks import make_identity
    from concourse import library_config

    F32, F16, I16, I32 = (
        mybir.dt.float32, mybir.dt.float16, mybir.dt.int16, mybir.dt.int32,
    )
    AF = mybir.ActivationFunctionType
    OP = mybir.AluOpType
    AX = mybir.AxisListType
    P = _P

    ncore = cfg["n_cores"]
    Bfull = cfg["B"]
    Bc = Bfull // ncore          # 2048 rows per core
    TB = Bc // P                 # 16 batch tiles per core
    NB = min(512, Bc)            # matmul moving free dim
    NN = Bc // NB                # 4
    TPN = NB // P                # 4
    NPAIR = F_CAT // 2           # 13 feature-pair K chunks
    NKC = NPAIR + 1              # + cont chunk
    NM1 = H1 // P                # 8
    NM2 = H2 // P                # 4
    rg = [list(range(ncore))]
    NQ = 4

    nc = bacc.Bacc(num_devices=ncore, num_swdge_queues=NQ)

    idxg = nc.dram_tensor("idxg", [P, F_CAT * IDXF], I16, kind="ExternalInput")
    parin = nc.dram_tensor("parin", [P, F_CAT * TB], F16, kind="ExternalInput")
    tabp = nc.dram_tensor("tabp", [F_CAT * VH, EP], F16, kind="ExternalInput")
    cfin = nc.dram_tensor("cfin", [Bc, F_CONT], F32, kind="ExternalInput")
    w1 = nc.dram_tensor("w1", [NKC * P, H1], F16, kind="ExternalInput")
    w2 = nc.dram_tensor("w2", [H1, H2], F16, kind="ExternalInput")
    w3 = nc.dram_tensor("w3", [P, NM2], F16, kind="ExternalInput")
    ct2 = nc.dram_tensor("ct2", [F_CONT, D], F32, kind="ExternalInput")
    cmisc = nc.dram_tensor("cmisc", [P, 2 * F_CONT], F32, kind="ExternalInput")
    bnp = nc.dram_tensor("bnp", [P, 3 * NM1 + 3 * NM2 + 1], F32, kind="ExternalInput")
    out = nc.dram_tensor("out", [P, 2 * TB], F32, kind="ExternalOutput")

    # gather plan per feature: (idx_off_in_f, NI, dst_slots, chunk0, sel_slots, sel_nch)
    GATHERS = [
        (0, 1024, 8, 0, 0, 7),     # A: samples 0..1023   -> chunks 0..7, read 0..6
        (64, 1024, 8, 7, 0, 7),    # B: samples 896..1919 -> chunks 7..14, read 7..13
        (128, 384, 3, 14, 0, 2),   # C: samples 1792..2047+pad -> 14..16, read 14..15
    ]

    with tile.TileContext(nc) as tc:
        with (
            tc.tile_pool(name="const", bufs=1) as cpool,
            tc.tile_pool(name="big", bufs=1) as bpool,
            tc.tile_pool(name="gat", bufs=3) as gpool,
            tc.tile_pool(name="gatc", bufs=3) as gcpool,
            tc.tile_pool(name="work", bufs=2) as wpool,
            tc.tile_pool(name="psmm", bufs=4, space="PSUM") as psmm,
            tc.tile_pool(name="pssm", bufs=1, space="PSUM") as pssm,
            tc.tile_pool(name="dram", bufs=1, space="DRAM") as dpool,
        ):
            nc.gpsimd.load_library(library_config.mlp)

            # ---- constants ----
            idx_sb = cpool.tile([P, F_CAT * IDXF], I16, tag="idxsb")
            nc.sync.dma_start(out=idx_sb[:], in_=idxg[:])
            par_sb = cpool.tile([P, F_CAT * TB], F16, tag="parsb")
            nc.sync.dma_start(out=par_sb[:], in_=parin[:])
            w2sb = []
            for k in range(NM1):
                t = cpool.tile([P, H2], F16, tag=f"w2_{k}")
                nc.sync.dma_start(out=t[:], in_=w2[k * P : (k + 1) * P, :])
                w2sb.append(t)
            w3sb = cpool.tile([P, NM2], F16, tag="w3")
            nc.sync.dma_start(out=w3sb[:], in_=w3[:])
            ct2sb = cpool.tile([F_CONT, D], F32, tag="ct2")
            nc.sync.dma_start(out=ct2sb[:], in_=ct2[:])
            cmsb = cpool.tile([P, 2 * F_CONT], F32, tag="cmisc")
            nc.sync.dma_start(out=cmsb[:], in_=cmisc[:])
            bnsb = cpool.tile([P, 3 * NM1 + 3 * NM2 + 1], F32, tag="bnp")
            nc.sync.dma_start(out=bnsb[:], in_=bnp[:])
            ident = cpool.tile([P, P], F32, tag="ident")
            make_identity(nc, ident[:])
            eps_t = cpool.tile([P, 1], F32, tag="eps")
            nc.vector.memset(eps_t[:], BN_EPS)

            b1c = bnsb[:, 0:NM1]
            g1c = bnsb[:, NM1 : 2 * NM1]
            be1c = bnsb[:, 2 * NM1 : 3 * NM1]
            o2 = 3 * NM1
            b2c = bnsb[:, o2 : o2 + NM2]
            g2c = bnsb[:, o2 + NM2 : o2 + 2 * NM2]
            be2c = bnsb[:, o2 + 2 * NM2 : o2 + 3 * NM2]
            bias_col = bnsb[:, o2 + 3 * NM2 : o2 + 3 * NM2 + 1]
            t1b = cmsb[:, 0:F_CONT]
            rb = cmsb[:, F_CONT : 2 * F_CONT]

            # ---- persistent activations ----
            xtn = [
                bpool.tile([P, NKC, NB], F16, tag=f"xtn_{n}", name=f"xtn_{n}")
                for n in range(NN)
            ]
            for n in range(NN):
                nc.vector.memset(xtn[n][:, NPAIR, :], 0.0)
            ep = [
                bpool.tile([P, TB, 2, D], F16, tag=f"ep_{fp}", name=f"ep_{fp}")
                for fp in range(NPAIR)
            ]
            h1t = [bpool.tile([P, Bc], F16, tag=f"h1_{m}", name=f"h1_{m}") for m in range(NM1)]
            h2t = [bpool.tile([P, Bc], F16, tag=f"h2_{m}", name=f"h2_{m}") for m in range(NM2)]

            # FM accumulators (col per batch tile)
            qcat = bpool.tile([P, TB], F32, tag="qcat")
            q2t = bpool.tile([P, TB], F32, tag="q2t")
            qct = bpool.tile([P, TB], F32, tag="qct")
            f1t = bpool.tile([P, TB], F32, tag="f1t")
            fct = bpool.tile([P, TB], F32, tag="fct")
            fm_all = bpool.tile([P, TB], F32, tag="fm")
            s3 = bpool.tile([P, TB, D], F32, tag="s3")
            acc1 = bpool.tile([P, NM1 * NN], F32, tag="acc1")
            acc1s = bpool.tile([P, NM1 * NN], F32, tag="acc1s")
            acc2 = bpool.tile([P, NM2 * NN], F32, tag="acc2")
            acc2s = bpool.tile([P, NM2 * NN], F32, tag="acc2s")
            scrh = bpool.tile([P, NB], F16, tag="scrh")
            out_sb = bpool.tile([P, 2 * TB], F32, tag="outsb")

            nc.vector.memset(s3[:], 0.0)
            nc.vector.memset(qcat[:], 0.0)
            nc.vector.memset(f1t[:], 0.0)

            # ---- cont features: transpose into X.T cont chunk + FM terms ----
            for t in range(TB):
                cf_t = wpool.tile([P, F_CONT], F32, tag="cf")
                nc.sync.dma_start(out=cf_t[:], in_=cfin[t * P : (t + 1) * P, :])
                tr_ps = pssm.tile([F_CONT, P], F32, tag="sm")
                nc.tensor.transpose(out=tr_ps[:], in_=cf_t[:], identity=ident[:])
                cfT = wpool.tile([F_CONT, P], F32, tag="cfT")
                nc.vector.tensor_copy(out=cfT[:], in_=tr_ps[:])
                n, tp = t // TPN, t % TPN
                nc.vector.tensor_copy(
                    out=xtn[n][0:F_CONT, NPAIR, tp * P : (tp + 1) * P], in_=tr_ps[:]
                )
                # s_cont = cfT.T @ ct2 : [P, 64] added into s3
                ss_ps = pssm.tile([P, D], F32, tag="sc")
                nc.tensor.matmul(
                    out=ss_ps[:], lhsT=cfT[:], rhs=ct2sb[:], start=True, stop=True
                )
                nc.vector.tensor_tensor(
                    out=s3[:, t, :], in0=s3[:, t, :], in1=ss_ps[:], op=OP.add
                )
                # cont second-order qc = sum_f cf^2 * r ; first-order fc
                c13a = wpool.tile([P, F_CONT], F32, tag="c13a")
                c13b = wpool.tile([P, F_CONT], F32, tag="c13b")
                nc.vector.tensor_tensor(out=c13a[:], in0=cf_t[:], in1=rb, op=OP.mult)
                nc.vector.tensor_tensor(out=c13b[:], in0=c13a[:], in1=cf_t[:], op=OP.mult)
                nc.vector.tensor_reduce(
                    out=qct[:, t : t + 1], in_=c13b[:], axis=AX.X, op=OP.add
                )
                nc.vector.tensor_tensor(out=c13a[:], in0=cf_t[:], in1=t1b, op=OP.mult)
                nc.vector.tensor_reduce(
                    out=fct[:, t : t + 1], in_=c13a[:], axis=AX.X, op=OP.add
                )

            # ---- gather waves ----
            def emit_wave(gi):
                ioff, NI, nslot, chunk0, s0, nch = GATHERS[gi]
                for f in range(F_CAT):
                    fp_, fi = f // 2, f % 2
                    pool = gpool if nslot == 8 else gcpool
                    g = pool.tile([P, nslot, EP], F16, tag=f"g{nslot}", name=f"g_{gi}_{f}")
                    nc.gpsimd.dma_gather(
                        g[:],
                        tabp[f * VH + WOFF : f * VH + WOFF + 2, :],
                        idx_sb[:, f * IDXF + ioff : f * IDXF + ioff + NI // 16],
                        NI, NI, EP,
                        queue_num=f % NQ,
                    )
                    lo = g[:, s0 : s0 + nch, 0:D]
                    hi = g[:, s0 : s0 + nch, 128 : 128 + D]
                    m = par_sb[:, f * TB + chunk0 : f * TB + chunk0 + nch]
                    m3 = m.rearrange("p (c o) -> p c o", o=1).to_broadcast(
                        [P, nch, D]
                    )
                    tmp = wpool.tile([P, 7 * D], F16, tag="seltmp")
                    t3 = tmp[:].rearrange("p (c d) -> p c d", d=D)[:, 0:nch, :]
                    epd = ep[fp_][:, chunk0 : chunk0 + nch, fi, :]
                    nc.vector.tensor_tensor(out=t3, in0=hi, in1=lo, op=OP.subtract)
                    nc.vector.tensor_tensor(out=t3, in0=t3, in1=m3, op=OP.mult)
                    nc.vector.tensor_tensor(out=epd, in0=lo, in1=t3, op=OP.add)
                    # t1 select + accumulate into f1t
                    lo1 = g[:, s0 : s0 + nch, D : D + 1].rearrange("p c o -> p (c o)")
                    hi1 = g[:, s0 : s0 + nch, 128 + D : 128 + D + 1].rearrange(
                        "p c o -> p (c o)"
                    )
                    t1a = wpool.tile([P, TB], F32, tag="t1a")
                    nc.vector.tensor_tensor(
                        out=t1a[:, 0:nch], in0=hi1, in1=lo1, op=OP.subtract
                    )
                    nc.vector.tensor_tensor(
                        out=t1a[:, 0:nch], in0=t1a[:, 0:nch], in1=m, op=OP.mult
                    )
                    nc.vector.tensor_tensor(
                        out=t1a[:, 0:nch], in0=t1a[:, 0:nch], in1=lo1, op=OP.add
                    )
                    nc.vector.tensor_tensor(
                        out=f1t[:, chunk0 : chunk0 + nch],
                        in0=f1t[:, chunk0 : chunk0 + nch],
                        in1=t1a[:, 0:nch],
                        op=OP.add,
                    )
                    # qcat += sum_d E^2 ; s3 += E
                    sq = wpool.tile([P, 7 * D], F32, tag="sqscr")
                    sq3 = sq[:].rearrange("p (c d) -> p c d", d=D)[:, 0:nch, :]
                    nc.scalar.activation(out=sq3, in_=epd, func=AF.Square)
                    qtmp = wpool.tile([P, TB], F32, tag="qtmp")
                    nc.vector.tensor_reduce(
                        out=qtmp[:, 0:nch], in_=sq3, axis=AX.X, op=OP.add
                    )
                    nc.vector.tensor_tensor(
                        out=qcat[:, chunk0 : chunk0 + nch],
                        in0=qcat[:, chunk0 : chunk0 + nch],
                        in1=qtmp[:, 0:nch],
                        op=OP.add,
                    )
                    nc.vector.tensor_tensor(
                        out=s3[:, chunk0 : chunk0 + nch, :],
                        in0=s3[:, chunk0 : chunk0 + nch, :],
                        in1=epd,
                        op=OP.add,
                    )
                    # transposes into X.T once both features of the pair are in
                    if fi == 1:
                        for n in ([0] if gi == 0 else [1, 2] if gi == 1 else [3]):
                            nc.sync.dma_start_transpose(
                                out=xtn[n][:, fp_, :].rearrange(
                                    "p (b c) -> p b c", c=P
                                ),
                                in_=ep[fp_][:, 4 * n : 4 * n + 4, :, :],
                            )

            for gi in range(3):
                emit_wave(gi)

            # fm = 0.5*(q2 - qcat - qct) + f1 + fc ; q2 from s3
            for tg in range(4):
                sqf = wpool.tile([P, 7 * D], F32, tag="sqscr")
                sqf3 = sqf[:].rearrange("p (c d) -> p c d", d=D)[:, 0:4, :]
                nc.scalar.activation(
                    out=sqf3, in_=s3[:, 4 * tg : 4 * tg + 4, :], func=AF.Square
                )
                nc.vector.tensor_reduce(
                    out=q2t[:, 4 * tg : 4 * tg + 4], in_=sqf3, axis=AX.X, op=OP.add
                )
            nc.vector.tensor_tensor(out=fm_all[:], in0=qcat[:], in1=qct[:], op=OP.add)
            nc.vector.tensor_tensor(out=fm_all[:], in0=q2t[:], in1=fm_all[:], op=OP.subtract)
            nc.vector.tensor_scalar(
                out=fm_all[:], in0=fm_all[:], scalar1=0.5, scalar2=None, op0=OP.mult
            )
            nc.vector.tensor_tensor(out=fm_all[:], in0=fm_all[:], in1=f1t[:], op=OP.add)
            nc.vector.tensor_tensor(out=fm_all[:], in0=fm_all[:], in1=fct[:], op=OP.add)

            # ---- layer 1 matmul (W1 streamed in [128,512] slices) ----
            for n in range(NN):
                for mh in range(2):
                    ps = [psmm.tile([P, NB], F32, tag="mm", name=f"ps_{n}_{mh}_{i}") for i in range(4)]
                    for k in range(NKC):
                        w1t = wpool.tile([P, H1 // 2], F16, tag="w1s")
                        nc.sync.dma_start(
                            out=w1t[:],
                            in_=w1[k * P : (k + 1) * P, mh * 512 : (mh + 1) * 512],
                        )
                        for mm in range(4):
                            nc.tensor.matmul(
                                out=ps[mm][:],
                                lhsT=w1t[:, mm * P : (mm + 1) * P],
                                rhs=xtn[n][:, k, :],
                                start=(k == 0),
                                stop=(k == NKC - 1),
                            )
                    for mm in range(4):
                        m = mh * 4 + mm
                        j = m * NN + n
                        nc.scalar.activation(
                            out=h1t[m][:, n * NB : (n + 1) * NB], in_=ps[mm][:],
                            func=AF.Identity, bias=b1c[:, m : m + 1],
                            accum_out=acc1[:, j : j + 1],
                        )
                        nc.scalar.activation(
                            out=scrh[:, :NB], in_=h1t[m][:, n * NB : (n + 1) * NB],
                            func=AF.Square,
                            accum_out=acc1s[:, j : j + 1],
                        )

            # ---- BN1 (AllReduce stats) ----
            st1 = bpool.tile([P, 2 * NM1], F32, tag="st1")
            nc.vector.tensor_reduce(
                out=st1[:, :NM1],
                in_=acc1[:].rearrange("p (m n) -> p m n", n=NN),
                axis=AX.X, op=OP.add,
            )
            nc.vector.tensor_reduce(
                out=st1[:, NM1:],
                in_=acc1s[:].rearrange("p (m n) -> p m n", n=NN),
                axis=AX.X, op=OP.add,
            )
            st1i = dpool.tile([P, 2 * NM1], F32, tag="st1i")
            st1o = dpool.tile([P, 2 * NM1], F32, tag="st1o")
            nc.gpsimd.dma_start(out=st1i[:], in_=st1[:])
            nc.gpsimd.collective_compute(
                "AllReduce", OP.add, replica_groups=rg,
                ins=[st1i[:].opt()], outs=[st1o[:].opt()],
            )
            gst1 = bpool.tile([P, 2 * NM1], F32, tag="gst1")
            nc.gpsimd.dma_start(out=gst1[:], in_=st1o[:])

            mu1 = bpool.tile([P, NM1], F32, tag="mu1")
            var1 = bpool.tile([P, NM1], F32, tag="var1")
            a1 = bpool.tile([P, NM1], F32, tag="a1")
            bp1 = bpool.tile([P, NM1], F32, tag="bp1")
            inv_b = 1.0 / Bfull
            nc.vector.tensor_scalar(
                out=mu1[:], in0=gst1[:, :NM1], scalar1=inv_b, scalar2=None, op0=OP.mult
            )
            nc.vector.tensor_tensor(out=var1[:], in0=mu1[:], in1=mu1[:], op=OP.mult)
            nc.vector.tensor_scalar(
                out=a1[:], in0=gst1[:, NM1:], scalar1=inv_b, scalar2=None, op0=OP.mult
            )
            nc.vector.tensor_tensor(out=var1[:], in0=a1[:], in1=var1[:], op=OP.subtract)
            nc.scalar.activation(
                out=var1[:], in_=var1[:], func=AF.Sqrt, bias=eps_t[:, 0:1]
            )
            nc.vector.reciprocal(out=var1[:], in_=var1[:])
            nc.vector.tensor_tensor(out=a1[:], in0=g1c, in1=var1[:], op=OP.mult)
            nc.vector.tensor_tensor(out=bp1[:], in0=mu1[:], in1=a1[:], op=OP.mult)
            nc.vector.tensor_tensor(out=bp1[:], in0=be1c, in1=bp1[:], op=OP.subtract)
            for m in range(NM1):
                nc.scalar.activation(
                    out=h1t[m][:], in_=h1t[m][:], func=AF.Relu,
                    scale=a1[:, m : m + 1], bias=bp1[:, m : m + 1],
                )

            # ---- layer 2 ----
            for n in range(NN):
                for m in range(NM2):
                    ps2 = psmm.tile([P, NB], F32, tag="mm")
                    for k in range(NM1):
                        nc.tensor.matmul(
                            out=ps2[:],
                            lhsT=w2sb[k][:, m * P : (m + 1) * P],
                            rhs=h1t[k][:, n * NB : (n + 1) * NB],
                            start=(k == 0),
                            stop=(k == NM1 - 1),
                        )
                    j = m * NN + n
                    nc.scalar.activation(
                        out=h2t[m][:, n * NB : (n + 1) * NB], in_=ps2[:],
                        func=AF.Identity, bias=b2c[:, m : m + 1],
                        accum_out=acc2[:, j : j + 1],
                    )
                    nc.scalar.activation(
                        out=scrh[:, :NB], in_=h2t[m][:, n * NB : (n + 1) * NB],
                        func=AF.Square,
                        accum_out=acc2s[:, j : j + 1],
                    )

            # ---- BN2 ----
            st2 = bpool.tile([P, 2 * NM2], F32, tag="st2")
            nc.vector.tensor_reduce(
                out=st2[:, :NM2],
                in_=acc2[:].rearrange("p (m n) -> p m n", n=NN),
                axis=AX.X, op=OP.add,
            )
            nc.vector.tensor_reduce(
                out=st2[:, NM2:],
                in_=acc2s[:].rearrange("p (m n) -> p m n", n=NN),
                axis=AX.X, op=OP.add,
            )
            st2i = dpool.tile([P, 2 * NM2], F32, tag="st2i")
            st2o = dpool.tile([P, 2 * NM2], F32, tag="st2o")
            nc.gpsimd.dma_start(out=st2i[:], in_=st2[:])
            nc.gpsimd.collective_compute(
                "AllReduce", OP.add, replica_groups=rg,
                ins=[st2i[:].opt()], outs=[st2o[:].opt()],
            )
            gst2 = bpool.tile([P, 2 * NM2], F32, tag="gst2")
            nc.gpsimd.dma_start(out=gst2[:], in_=st2o[:])

            mu2 = bpool.tile([P, NM2], F32, tag="mu2")
            var2 = bpool.tile([P, NM2], F32, tag="var2")
            a2 = bpool.tile([P, NM2], F32, tag="a2")
            bp2 = bpool.tile([P, NM2], F32, tag="bp2")
            nc.vector.tensor_scalar(
                out=mu2[:], in0=gst2[:, :NM2], scalar1=inv_b, scalar2=None, op0=OP.mult
            )
            nc.vector.tensor_tensor(out=var2[:], in0=mu2[:], in1=mu2[:], op=OP.mult)
            nc.vector.tensor_scalar(
                out=a2[:], in0=gst2[:, NM2:], scalar1=inv_b, scalar2=None, op0=OP.mult
            )
            nc.vector.tensor_tensor(out=var2[:], in0=a2[:], in1=var2[:], op=OP.subtract)
            nc.scalar.activation(
                out=var2[:], in_=var2[:], func=AF.Sqrt, bias=eps_t[:, 0:1]
            )
            nc.vector.reciprocal(out=var2[:], in_=var2[:])
            nc.vector.tensor_tensor(out=a2[:], in0=g2c, in1=var2[:], op=OP.mult)
            nc.vector.tensor_tensor(out=bp2[:], in0=mu2[:], in1=a2[:], op=OP.mult)
            nc.vector.tensor_tensor(out=bp2[:], in0=be2c, in1=bp2[:], op=OP.subtract)
            for m in range(NM2):
                nc.scalar.activation(
                    out=h2t[m][:], in_=h2t[m][:], func=AF.Relu,
                    scale=a2[:, m : m + 1], bias=bp2[:, m : m + 1],
                )

            # ---- layer 3 + sigmoid + output ----
            for t in range(TB):
                psd = pssm.tile([P, 1], F32, tag="d3")
                for c in range(NM2):
                    nc.tensor.matmul(
                        out=psd[:],
                        lhsT=h2t[c][:, t * P : (t + 1) * P],
                        rhs=w3sb[:, c : c + 1],
                        start=(c == 0),
                        stop=(c == NM2 - 1),
                    )
                zt = wpool.tile([P, 1], F32, tag="zt")
                nc.vector.tensor_tensor(
                    out=zt[:], in0=fm_all[:, t : t + 1], in1=psd[:], op=OP.add
                )
                nc.scalar.activation(
                    out=out_sb[:, 2 * t + 1 : 2 * t + 2], in_=zt[:],
                    func=AF.Sigmoid, bias=bias_col,
                )
                nc.scalar.activation(
                    out=out_sb[:, 2 * t : 2 * t + 1],
                    in_=out_sb[:, 2 * t + 1 : 2 * t + 2],
                    func=AF.Copy, bias=1.0, scale=-1.0,
                )
            nc.sync.dma_start(out=out[:], in_=out_sb[:])

    return nc


def _prep_shared(inputs, cfg):
    """Host-side parameter prep (batch-independent)."""
    f32, f16 = np.float32, np.float16
    cat_t1 = np.asarray(inputs["cat_t1"], f32)          # [26, V]
    cat_t2 = np.asarray(inputs["cat_t2"], f32)          # [26, V, 64]
    cont_t1 = np.asarray(inputs["cont_t1"], f32)        # [13]
    cont_t2 = np.asarray(inputs["cont_t2"], f32)        # [13, 64]
    W1 = np.asarray(inputs["W1"], f32)                  # [2496, 1024]
    W2 = np.asarray(inputs["W2"], f32)
    W3 = np.asarray(inputs["W3"], f32)                  # [512, 1]
    b1 = np.asarray(inputs["b1"], f32)
    g1 = np.asarray(inputs["g1"], f32)
    be1 = np.asarray(inputs["be1"], f32)
    b2 = np.asarray(inputs["b2"], f32)
    g2 = np.asarray(inputs["g2"], f32)
    be2 = np.asarray(inputs["be2"], f32)
    b3 = np.asarray(inputs["b3"], f32)
    bias = np.asarray(inputs["bias"], f32)

    # packed fp16 pair-row table: [emb(64) | t1 | pad(63)] x 2 entries
    tab4 = np.zeros((F_CAT, VH, 2, 128), f16)
    tab4[..., :D] = cat_t2.reshape(F_CAT, VH, 2, D)
    tab4[..., D] = cat_t1.reshape(F_CAT, VH, 2)
    tabp = tab4.reshape(F_CAT * VH, EP)

    ncat = F_CAT * D  # 1664
    NKC = ncat // _P + 1
    W1eff = np.einsum("fd,fdh->fh", cont_t2, W1[ncat:].reshape(F_CONT, D, H1))
    w1p = np.zeros((NKC * _P, H1), f32)
    w1p[:ncat] = W1[:ncat]
    w1p[ncat : ncat + F_CONT] = W1eff

    NM1, NM2 = H1 // _P, H2 // _P
    bnp = np.zeros((_P, 3 * NM1 + 3 * NM2 + 1), f32)
    bnp[:, 0:NM1] = b1.reshape(NM1, _P).T
    bnp[:, NM1 : 2 * NM1] = g1.reshape(NM1, _P).T
    bnp[:, 2 * NM1 : 3 * NM1] = be1.reshape(NM1, _P).T
    o2 = 3 * NM1
    bnp[:, o2 : o2 + NM2] = b2.reshape(NM2, _P).T
    bnp[:, o2 + NM2 : o2 + 2 * NM2] = g2.reshape(NM2, _P).T
    bnp[:, o2 + 2 * NM2 : o2 + 3 * NM2] = be2.reshape(NM2, _P).T
    bnp[:, o2 + 3 * NM2] = float(bias[0]) + float(b3[0])

    cmisc = np.zeros((_P, 2 * F_CONT), f32)
    cmisc[:, :F_CONT] = cont_t1[None, :]
    cmisc[:, F_CONT:] = (cont_t2**2).sum(axis=1)[None, :]

    return {
        "tabp": tabp,
        "w1": w1p.astype(f16),
        "w2": W2.astype(f16),
        "w3": W3[:, 0].reshape(NM2, _P).T.astype(f16).copy(),
        "ct2": cont_t2,
        "cmisc": cmisc,
        "bnp": bnp,
    }


def _prep_idx(cat_core):
    """Per-core gather index blocks + parity masks.

    cat_core: int64 [2048, 26]. Returns (idx_sb [128, 26*IDXF] i16,
    par_sb [128, 26*16] f16).
    """
    Bc = cat_core.shape[0]
    u = (cat_core >> 1).astype(np.int32) - WOFF      # [2048, 26]
    idxp = u.astype(np.int16)
    par = (cat_core & 1).astype(np.float16)          # [2048, 26]

    blocks = np.zeros((F_CAT, IDXF, 16), np.int16)
    plans = [(0, 0, 1024, 64), (64, 896, 1024, 64), (128, 1792, 256, 24)]
    for off, s0, nreal, nfree in plans:
        NI = nfree * 16
        for f in range(F_CAT):
            lst = np.full(NI, 1, np.int16)
            lst[:nreal] = idxp[s0 : s0 + nreal, f]
            blocks[f, off : off + nfree, :] = lst.reshape(nfree, 16)
    idx_sb = np.tile(blocks.transpose(2, 0, 1).reshape(16, F_CAT * IDXF), (8, 1))

    TB = Bc // _P
    par_sb = (
        par.reshape(TB, _P, F_CAT).transpose(1, 2, 0).reshape(_P, F_CAT * TB).copy()
    )
    return idx_sb, par_sb


def _prep_in_maps(inputs, cfg):
    ncore = cfg["n_cores"]
    Bc = cfg["B"] // ncore
    shared = _prep_shared(inputs, cfg)
    cat = np.asarray(inputs["cat_feats"]).astype(np.int64)
    cont = np.asarray(inputs["cont_feats"], np.float32)
    in_maps = []
    for c in range(ncore):
        m = dict(shared)
        idx_sb, par_sb = _prep_idx(cat[c * Bc : (c + 1) * Bc])
        m["idxg"] = idx_sb
        m["parin"] = par_sb
        m["cfin"] = cont[c * Bc : (c + 1) * Bc]
        in_maps.append(m)
    return in_maps


def _unshard(results, cfg):
    ncore = cfg["n_cores"]
    Bc = cfg["B"] // ncore
    TB = Bc // _P
    outs = []
    for c in range(ncore):
        a = results[c]["out"]  # [128, 2*TB]
        outs.append(a.reshape(_P, TB, 2).transpose(1, 0, 2).reshape(Bc, 2))
    return np.concatenate(outs, axis=0)


_CACHE = {}


def _get_program(cfg_key):
    if cfg_key not in _CACHE:
        cfg = dict(B=cfg_key[0], V=cfg_key[1], n_cores=cfg_key[2])
        nc = _build_program(cfg)
        nc.finalize()
        _CACHE[cfg_key] = nc
    return _CACHE[cfg_key]


def run(inputs, trace=False, cfg=None):
    from concourse import bass_utils

    cfg = cfg or CFG_FULL
    nc = _get_program((cfg["B"], cfg["V"], cfg["n_cores"]))
    in_maps = _prep_in_maps(inputs, cfg)
    res = bass_utils.run_bass_kernel_spmd(
        nc, in_maps, core_ids=list(range(cfg["n_cores"])), trace=trace
    )
    return _unshard(res.results, cfg), res


def kernel(**inputs) -> np.ndarray:
    out, _ = run(inputs, trace=False)
    return out
